# revision 32
# baseline (speedup 1.0000x reference)
"""Top-1 MoE block (B=4, S=2048, H=2048, E=8) for 8 Trainium2 NeuronCores.

Strategy (expert-parallel, host-mediated dispatch):
  - Host computes the tiny gating network (x @ Wg -> softmax -> argmax),
    0.4% of total FLOPs, and the token permutation per expert.
  - Token block for expert e (prob-scaled, cast to bf16, tiled) plus
    W[e] (bf16) goes to core e.  Each core runs a dense matmul in bf16
    (full PE rate, half the HBM traffic of fp32).
  - Tokens beyond 1024 per expert ("overflow", ~210 of 8192 here) are
    packed into one extra half-contraction tile per core: a core pair
    (2g, 2g+1) computes K-halves [0:1024) / [1024:2048) of overflow
    group g; the host sums the two partial outputs.  This keeps every
    core at 8 full m-tiles + 1 half-K tile instead of 9 full tiles.
  - Host upcasts per-expert outputs, scatters back to token order.

Schedule per core:
  - gpsimd (SWDGE) loads x tiles 0,1 while the two HWDGE queues
    (sync, scalar) stream W k-tiles even/odd as 0.5MB contiguous DMAs.
  - Tensor engine pre-warms the PE (HAM un-throttle) with junk matmuls
    while the first DMAs land, then chases the W stream k-major over
    m-tiles {0,1} (m-tile 1 lagging 4 k-tiles so its x and the m-tile-0
    PSUM copy stay off the critical path), then streams the remaining
    m-tiles with W resident.  The final (overflow) tile runs nt-major
    so its casts+stores pipeline behind the last matmuls.
  - Vector (+gpsimd for the final slices) casts PSUM fp32 -> bf16;
    sync+scalar issue the stores.
"""

import os

import numpy as np
import ml_dtypes

import concourse.bass as bass
from concourse import mybir
from concourse.bass_utils import run_bass_kernel_spmd

BF16 = ml_dtypes.bfloat16

B, S, H, E = 4, 2048, 2048, 8
P = 128
KT = H // P  # 16 k tiles
N_FREE = 512  # matmul moving free dim / PSUM bank width (fp32)
NT = H // N_FREE  # 4 n tiles
N_CORES = 8
LAG = 2  # phase-1 m-tile-1 lag (k-tiles)
N_WARM = 30  # pre-warm matmuls, N=256 (~213ns cold / ~110ns warm each)
WARM_N = 256
CAP = 1024  # main-tile token capacity per core in overflow mode

_COMPILED = {}


def _ensure_ntff_hook() -> bool:
    """Register antenv.axon_hooks with a ctypes NTFF hook if the image lacks it."""
    import contextlib
    import ctypes
    import sys
    import types

    try:
        from antenv.axon_hooks import get_axon_ntff_profile_hook  # noqa: F401

        return True
    except ImportError:
        pass

    so_path = "/opt/axon/libaxon_pjrt.so"
    if not os.path.exists(so_path):
        return False
    lib = ctypes.CDLL(so_path)
    if not hasattr(lib, "axon_start_nrt_profile"):
        return False
    lib.axon_start_nrt_profile.argtypes = [
        ctypes.POINTER(ctypes.c_int64),
        ctypes.c_size_t,
    ]
    lib.axon_start_nrt_profile.restype = ctypes.c_int64
    lib.axon_stop_nrt_profile.argtypes = [ctypes.c_char_p]
    lib.axon_stop_nrt_profile.restype = ctypes.c_int64

    @contextlib.contextmanager
    def _hook(output_dir, device_ids):
        import jax

        jax.devices()  # force PJRT init so the .so's client exists
        if device_ids:
            ids = (ctypes.c_int64 * len(device_ids))(*device_ids)
            rc = lib.axon_start_nrt_profile(ids, len(device_ids))
        else:
            rc = lib.axon_start_nrt_profile(None, 0)
        if rc != 0:
            raise RuntimeError(f"axon_start_nrt_profile rc={rc}")
        try:
            yield
        finally:
            n = lib.axon_stop_nrt_profile(str(output_dir).encode())
            print(f"ntff profile: {n} file(s) -> {output_dir}")

    import antenv

    mod = types.ModuleType("antenv.axon_hooks")
    mod.get_axon_ntff_profile_hook = lambda: _hook
    mod.set_axon_ntff_profile_hook = lambda h: None
    sys.modules["antenv.axon_hooks"] = mod
    antenv.axon_hooks = mod
    return True


def _build_bass(n_main: int, ov: bool) -> bass.Bass:
    """SPMD kernel for one core.

    Main tiles: y[mt] = xt[mt].T @ w for mt in 0..n_main-1 (full K=2048).
    Final tile: ov=True  -> y2 = xt2.T @ w2 with K=1024 (overflow half).
                ov=False -> the last main m-tile (mt = n_main, full K),
                            run nt-major for tail pipelining.

    xt: [MTx*128, KT*128] where row mt*128+p, col kt*128+t holds
        x_token[mt*128+t, kt*128+p] (per-m-tile transposed blocks).
    w:  [H, H] row-major.  y: [MTx*128, H] bf16.
    xt2: [128, 1024], w2: [1024, H], y2: [128, H] (ov mode only).
    """
    assert n_main >= 3
    f32 = mybir.dt.float32
    bf16 = mybir.dt.bfloat16
    MTx = n_main if ov else n_main + 1  # m-tiles in xt/y
    KTF = (H // 2 if ov else H) // P  # k-tiles of the final tile

    nc = bass.Bass()
    xt = nc.dram_tensor("xt", [MTx * P, KT * P], bf16, kind="ExternalInput")
    w = nc.dram_tensor("w", [H, H], bf16, kind="ExternalInput")
    y = nc.dram_tensor("y", [MTx * P, H], bf16, kind="ExternalOutput")
    if ov:
        xt2 = nc.dram_tensor("xt2", [P, KTF * P], bf16, kind="ExternalInput")
        w2 = nc.dram_tensor("w2", [KTF * P, H], bf16, kind="ExternalInput")
        y2 = nc.dram_tensor("y2", [P, H], bf16, kind="ExternalOutput")

    with (
        nc.sbuf_tensor("w_sb", [P, KT, H], bf16) as w_sb,
        nc.sbuf_tensor("x_sb", [P, n_main, H], bf16) as x_sb,
        nc.sbuf_tensor("y_sb", [P, n_main, H], bf16) as y_sb,
        nc.sbuf_tensor("xf_sb", [P, KTF * P], bf16) as xf_sb,
        nc.sbuf_tensor("yf_sb", [P, H], bf16) as yf_sb,
        nc.sbuf_tensor(
            "wf_sb", [P, KTF if ov else 1, H if ov else 2], bf16
        ) as wf_alloc,
        nc.sbuf_tensor("warm", [P, WARM_N], bf16) as warm,
        nc.psum_tensor("ps0", [P, H], f32) as ps0,
        nc.psum_tensor("ps1", [P, H], f32) as ps1,
        nc.semaphore("sPE") as sPE,
        nc.semaphore("sCopy") as sCopy,
        nc.semaphore("sWarm") as sWarm,
        nc.semaphore("sXf") as sXf,
        nc.semaphore("sCLv") as sCLv,
        nc.semaphore("sCLg") as sCLg,
        nc.semaphore("sYsync") as sYsync,
        nc.semaphore("sYscal") as sYscal,
        nc.semaphore("sW0b") as sW0b,
        nc.semaphore("sX0b") as sX0b,
        nc.Block() as block,
    ):
        psums = [ps0, ps1]
        sW = [nc.semaphore(f"sW{kt}").__enter__() for kt in range(KT)]
        sX = [nc.semaphore(f"sX{mt}").__enter__() for mt in range(n_main)]
        sY = [nc.semaphore(f"sY{mt}").__enter__() for mt in range(n_main)]
        ps_f = psums[n_main % 2]
        if ov:
            wf_sb = wf_alloc
            sWf = [nc.semaphore(f"sWf{kt}").__enter__() for kt in range(KTF)]
        else:
            wf_sb, sWf = w_sb, sW  # final tile reuses resident W

        USE_GPSIMD_X = os.environ.get("MOE_GPSIMD_X", "0") == "1"
        HHG = H // 2

        if USE_GPSIMD_X:

            @block.gpsimd
            def _(gp):
                # SWDGE: x tiles 0,1 in parallel with the W HWDGE streams.
                gp.dma_start(x_sb[:, 0, 0:HHG], xt[0:P, 0:HHG]).then_inc(
                    sX[0], 16
                )
                gp.dma_start(x_sb[:, 0, HHG:H], xt[0:P, HHG:H]).then_inc(
                    sX0b, 16
                )
                gp.dma_start(x_sb[:, 1, :], xt[P : 2 * P, :]).then_inc(sX[1], 16)
                if ov:
                    gp.dma_start(xf_sb[:, :], xt2[:, :]).then_inc(sXf, 16)
                else:
                    gp.dma_start(
                        xf_sb[:, :], xt[n_main * P : (n_main + 1) * P, :]
                    ).then_inc(sXf, 16)


        yf_dst = y2 if ov else y
        r0 = 0 if ov else n_main * P

        HH = H // 2

        @block.sync
        def _(sync):
            # W0 first-half + x0 first-half first (minimal bytes before the
            # first real matmul), then even W k-tiles, x tiles 2.., even w2
            # tiles, final stores 0,1,3
            sync.dma_start(w_sb[:, 0, 0:HH], w[0:P, 0:HH]).then_inc(sW[0], 16)
            if not USE_GPSIMD_X:
                sync.dma_start(x_sb[:, 0, 0:HH], xt[0:P, 0:HH]).then_inc(
                    sX[0], 16
                )
            for kt in range(2, KT, 2):
                sync.dma_start(
                    w_sb[:, kt, :], w[kt * P : (kt + 1) * P, :]
                ).then_inc(sW[kt], 16)
            for mt in range(2, n_main):
                sync.dma_start(
                    x_sb[:, mt, :], xt[mt * P : (mt + 1) * P, :]
                ).then_inc(sX[mt], 16)
            if not USE_GPSIMD_X:
                if ov:
                    sync.dma_start(xf_sb[:, :], xt2[:, :]).then_inc(sXf, 16)
                else:
                    sync.dma_start(
                        xf_sb[:, :], xt[n_main * P : (n_main + 1) * P, :]
                    ).then_inc(sXf, 16)
            if ov:
                for kt in range(0, KTF, 2):
                    sync.dma_start(
                        wf_sb[:, kt, :], w2[kt * P : (kt + 1) * P, :]
                    ).then_inc(sWf[kt], 16)
            for i, nt in enumerate((0, 1, 3)):
                sync.wait_ge(sCLv, i + 1)
                sync.dma_start(
                    yf_dst[r0 : r0 + P, nt * N_FREE : (nt + 1) * N_FREE],
                    yf_sb[:, nt * N_FREE : (nt + 1) * N_FREE],
                ).then_inc(sYsync, 16)
            sync.wait_ge(sYsync, 48)

        @block.scalar
        def _(scalar):
            # W0 second-half, W1, x1, x0 second-half, odd W k-tiles, odd w2
            # tiles, main stores, final store 2
            scalar.dma_start(w_sb[:, 0, HH:H], w[0:P, HH:H]).then_inc(
                sW0b, 16
            )
            scalar.dma_start(w_sb[:, 1, :], w[P : 2 * P, :]).then_inc(
                sW[1], 16
            )
            if not USE_GPSIMD_X:
                scalar.dma_start(x_sb[:, 1, :], xt[P : 2 * P, :]).then_inc(
                    sX[1], 16
                )
                scalar.dma_start(x_sb[:, 0, HH:H], xt[0:P, HH:H]).then_inc(
                    sX0b, 16
                )
            for kt in range(3, KT, 2):
                scalar.dma_start(
                    w_sb[:, kt, :], w[kt * P : (kt + 1) * P, :]
                ).then_inc(sW[kt], 16)
            if ov:
                for kt in range(1, KTF, 2):
                    scalar.dma_start(
                        wf_sb[:, kt, :], w2[kt * P : (kt + 1) * P, :]
                    ).then_inc(sWf[kt], 16)
            for mt in range(n_main):
                scalar.wait_ge(sCopy, mt + 1)
                scalar.dma_start(
                    y[mt * P : (mt + 1) * P, :], y_sb[:, mt, :]
                ).then_inc(sY[mt], 16)
            for i, nt in enumerate((2,)):
                # ACT casts its own slice then stores it; the sem wait
                # orders the DMA behind the copy's SBUF writes (same-engine
                # issue does NOT imply write completion).
                scalar.wait_ge(sPE, n_main + nt + 1)
                scalar.copy(
                    yf_sb[:, nt * N_FREE : (nt + 1) * N_FREE],
                    ps_f[:, nt * N_FREE : (nt + 1) * N_FREE],
                ).then_inc(sCLg, 1)
                scalar.wait_ge(sCLg, i + 1)
                scalar.dma_start(
                    yf_dst[r0 : r0 + P, nt * N_FREE : (nt + 1) * N_FREE],
                    yf_sb[:, nt * N_FREE : (nt + 1) * N_FREE],
                ).then_inc(sYscal, 16)
            for mt in range(n_main):
                scalar.wait_ge(sY[mt], 16)
            scalar.wait_ge(sYscal, 16)

        @block.tensor
        def _(tensor):
            def mm(psum, mt, kt, nt, start, stop):
                return tensor.matmul(
                    psum[:, nt * N_FREE : (nt + 1) * N_FREE],
                    x_sb[:, mt, kt * P : (kt + 1) * P],
                    w_sb[:, kt, nt * N_FREE : (nt + 1) * N_FREE],
                    start=start,
                    stop=stop,
                    skip_group_check=True,
                )

            def mm4(psum, mt, kt):
                for nt in range(NT):
                    m = mm(psum, mt, kt, nt, kt == 0, kt == KT - 1)
                return m

            # Pre-warm the PE (HAM un-throttles after ~3.4us of activity)
            # on scratch data while the first DMAs land.
            tensor.wait_ge(sWarm, 1)
            for _ in range(N_WARM):
                tensor.matmul(
                    ps0[:, 0:WARM_N],
                    warm[:, 0:P],
                    warm[:, :],
                    start=True,
                    stop=True,
                    skip_group_check=True,
                )

            # Phase 1: m-tiles 0,1 k-major chasing the W DMA streams,
            # m-tile 1 lagging LAG k-tiles.
            tensor.wait_ge(sX[0], 16)
            for kt in range(KT):
                tensor.wait_ge(sW[kt], 16)
                if kt == 0:
                    # W0/x0 second halves stream in behind the first halves
                    for nt in (0, 1):
                        mm(ps0, 0, 0, nt, True, False)
                    tensor.wait_ge(sW0b, 16)
                    for nt in (2, 3):
                        m = mm(ps0, 0, 0, nt, True, False)
                else:
                    if kt == KT // 2:
                        tensor.wait_ge(sX0b, 16)
                    m = mm4(ps0, 0, kt)
                if kt == KT - 1:
                    m.then_inc(sPE, 1)
                if kt == LAG:
                    tensor.wait_ge(sX[1], 16)
                if kt >= LAG:
                    mm4(ps1, 1, kt - LAG)
            for kt in range(KT - LAG, KT):
                m = mm4(ps1, 1, kt)
            m.then_inc(sPE, 1)
            # Phase 2: W resident; stream the remaining m-tiles.
            for mt in range(2, n_main):
                tensor.wait_ge(sX[mt], 16)
                tensor.wait_ge(sCopy, mt - 1)  # psum slot free
                for kt in range(KT):
                    m = mm4(psums[mt % 2], mt, kt)
                m.then_inc(sPE, 1)
            # Final tile: nt-major so each col-slice finishes early and
            # its cast+store pipelines behind the remaining matmuls.
            tensor.wait_ge(sXf, 16)
            tensor.wait_ge(sCopy, n_main - 1)
            for nt in range(NT):
                for kt in range(KTF):
                    if nt == 0:
                        tensor.wait_ge(sWf[kt], 16)
                    m = tensor.matmul(
                        ps_f[:, nt * N_FREE : (nt + 1) * N_FREE],
                        xf_sb[:, kt * P : (kt + 1) * P],
                        wf_sb[:, kt, nt * N_FREE : (nt + 1) * N_FREE],
                        start=(kt == 0),
                        stop=(kt == KTF - 1),
                        skip_group_check=True,
                    )
                m.then_inc(sPE, 1)

        @block.vector
        def _(vector):
            vector.memset(warm[:, :], 0.25).then_inc(sWarm, 1)
            for mt in range(n_main):
                vector.wait_ge(sPE, mt + 1)
                vector.tensor_copy(
                    y_sb[:, mt, :], psums[mt % 2][:, :]
                ).then_inc(sCopy, 1)
            for nt in (0, 1, 3):
                vector.wait_ge(sPE, n_main + nt + 1)
                vector.tensor_copy(
                    yf_sb[:, nt * N_FREE : (nt + 1) * N_FREE],
                    ps_f[:, nt * N_FREE : (nt + 1) * N_FREE],
                ).then_inc(sCLv, 1)

    return nc


def _route(x, Wg):
    """Host gating: returns token indices per expert and top-1 probs."""
    xf = np.ascontiguousarray(x.reshape(-1, H))
    logits = xf @ Wg  # [T, E] fp32 (min top1-top2 gap ~1e-4)
    idx = logits.argmax(-1)
    m = logits.max(-1, keepdims=True)
    ex = np.exp(logits - m)
    p = (ex[np.arange(len(idx)), idx] / ex.sum(-1)).astype(np.float32)
    return xf, idx, p


def _pack_tiles(xs: np.ndarray, n_tiles: int, k: int) -> np.ndarray:
    """[n_tok, k] fp32 -> [n_tiles*128, k] bf16 per-m-tile transposed tiles.

    Row mt*128+p, col kt*128+t  <-  xs[mt*128+t, kt*128+p].
    """
    n = xs.shape[0]
    kt = k // P
    out = np.zeros((n_tiles * P, k), dtype=BF16)
    for mt in range(n_tiles):
        t0, t1 = mt * P, min((mt + 1) * P, n)
        if t0 >= t1:
            break
        blk = xs[t0:t1].astype(BF16)  # [tc, k]
        tc = t1 - t0
        dst = out[mt * P : (mt + 1) * P].reshape(P, kt, P)  # [p, kt, t]
        dst[:, :, :tc] = blk.reshape(tc, kt, P).transpose(2, 1, 0)
    return out


def _run(inputs, trace=False):
    x = np.asarray(inputs["x"], dtype=np.float32)
    Wg = np.asarray(inputs["Wg"], dtype=np.float32)
    W = np.asarray(inputs["W"], dtype=np.float32)
    b = np.asarray(inputs["b"], dtype=np.float32)

    if trace:
        trace = _ensure_ntff_hook()

    xf, idx, p = _route(x, Wg)
    T = xf.shape[0]

    toks = [np.nonzero(idx == e)[0] for e in range(E)]
    counts = np.array([len(t) for t in toks])

    # Overflow pieces: per-expert token chunks beyond CAP, each <= 128.
    pieces = []
    for e in range(E):
        o = toks[e][CAP:]
        for i in range(0, len(o), P):
            pieces.append((e, o[i : i + P]))

    ov = 0 < len(pieces) <= N_CORES // 2 and counts.max() > CAP
    if ov:
        n_main = CAP // P
        key = ("OV", n_main)
    else:
        n_main = max(3, int(-(-counts.max() // P)) - 1)
        key = ("A", n_main)
    if key not in _COMPILED:
        _COMPILED[key] = _build_bass(n_main, ov)
    nc = _COMPILED[key]

    KH = H // 2
    in_maps = []
    for c in range(N_CORES):
        e = c
        te = toks[e][: CAP if ov else None]
        xs = xf[te] * p[te, None]  # fold gate prob into activations
        m = {
            "xt": _pack_tiles(xs, n_main if ov else n_main + 1, H),
            "w": W[e].astype(BF16),
        }
        if ov:
            g, h = c // 2, c % 2
            if g < len(pieces):
                e2, t2 = pieces[g]
                xs2 = (xf[t2] * p[t2, None])[:, h * KH : (h + 1) * KH]
                m["xt2"] = _pack_tiles(xs2, 1, KH)
                m["w2"] = W[e2][h * KH : (h + 1) * KH].astype(BF16)
            else:
                m["xt2"] = np.zeros((P, KH), dtype=BF16)
                m["w2"] = np.zeros((KH, H), dtype=BF16)
        in_maps.append(m)

    res = run_bass_kernel_spmd(
        nc,
        in_maps,
        core_ids=list(range(N_CORES)),
        trace=trace,
        trace_cores=list(range(N_CORES)) if trace else None,
    )

    out = np.empty((T, H), dtype=np.float32)
    for e in range(E):
        te = toks[e][: CAP if ov else None]
        ye = res.results[e]["y"][: len(te)].astype(np.float32)
        if np.any(b[e]):
            ye = ye + p[te, None] * b[e]
        out[te] = ye
    if ov:
        for g in range(len(pieces)):
            e2, t2 = pieces[g]
            ye = (
                res.results[2 * g]["y2"][: len(t2)].astype(np.float32)
                + res.results[2 * g + 1]["y2"][: len(t2)].astype(np.float32)
            )
            if np.any(b[e2]):
                ye = ye + p[t2, None] * b[e2]
            out[t2] = ye
    return out.reshape(B, S, H), res


def kernel(**inputs) -> np.ndarray:
    out, _ = _run(inputs, trace=os.environ.get("MOE_TRACE", "0") == "1")
    return out


def run_traced(inputs):
    """For test.py: returns (output, BassKernelResults with exec_time_ns)."""
    return _run(inputs, trace=True)


# revision 34
# speedup vs baseline: 1.0090x; 1.0090x over previous
"""Top-1 MoE block (B=4, S=2048, H=2048, E=8) for 8 Trainium2 NeuronCores.

Strategy (expert-parallel, host-mediated dispatch):
  - Host computes the tiny gating network (x @ Wg -> softmax -> argmax),
    0.4% of total FLOPs, and the token permutation per expert.
  - Token block for expert e (prob-scaled, cast to bf16, tiled) plus
    W[e] (bf16) goes to core e.  Each core runs a dense matmul in bf16
    (full PE rate, half the HBM traffic of fp32).
  - Tokens beyond 1024 per expert ("overflow", ~210 of 8192 here) are
    packed into one extra half-contraction tile per core: a core pair
    (2g, 2g+1) computes K-halves [0:1024) / [1024:2048) of overflow
    group g; the host sums the two partial outputs.  This keeps every
    core at 8 full m-tiles + 1 half-K tile instead of 9 full tiles.
  - Host upcasts per-expert outputs, scatters back to token order.

Schedule per core:
  - gpsimd (SWDGE) loads x tiles 0,1 while the two HWDGE queues
    (sync, scalar) stream W k-tiles even/odd as 0.5MB contiguous DMAs.
  - Tensor engine pre-warms the PE (HAM un-throttle) with junk matmuls
    while the first DMAs land, then chases the W stream k-major over
    m-tiles {0,1} (m-tile 1 lagging 4 k-tiles so its x and the m-tile-0
    PSUM copy stay off the critical path), then streams the remaining
    m-tiles with W resident.  The final (overflow) tile runs nt-major
    so its casts+stores pipeline behind the last matmuls.
  - Vector (+gpsimd for the final slices) casts PSUM fp32 -> bf16;
    sync+scalar issue the stores.
"""

import os

import numpy as np
import ml_dtypes

import concourse.bass as bass
from concourse import mybir
from concourse.bass_utils import run_bass_kernel_spmd

BF16 = ml_dtypes.bfloat16

B, S, H, E = 4, 2048, 2048, 8
P = 128
KT = H // P  # 16 k tiles
N_FREE = 512  # matmul moving free dim / PSUM bank width (fp32)
NT = H // N_FREE  # 4 n tiles
N_CORES = 8
LAG = 2  # phase-1 m-tile-1 lag (k-tiles)
N_WARM = 30  # pre-warm matmuls, N=256 (~213ns cold / ~110ns warm each)
WARM_N = 256
CAP = 1024  # main-tile token capacity per core in overflow mode

_COMPILED = {}


def _ensure_ntff_hook() -> bool:
    """Register antenv.axon_hooks with a ctypes NTFF hook if the image lacks it."""
    import contextlib
    import ctypes
    import sys
    import types

    try:
        from antenv.axon_hooks import get_axon_ntff_profile_hook  # noqa: F401

        return True
    except ImportError:
        pass

    so_path = "/opt/axon/libaxon_pjrt.so"
    if not os.path.exists(so_path):
        return False
    lib = ctypes.CDLL(so_path)
    if not hasattr(lib, "axon_start_nrt_profile"):
        return False
    lib.axon_start_nrt_profile.argtypes = [
        ctypes.POINTER(ctypes.c_int64),
        ctypes.c_size_t,
    ]
    lib.axon_start_nrt_profile.restype = ctypes.c_int64
    lib.axon_stop_nrt_profile.argtypes = [ctypes.c_char_p]
    lib.axon_stop_nrt_profile.restype = ctypes.c_int64

    @contextlib.contextmanager
    def _hook(output_dir, device_ids):
        import jax

        jax.devices()  # force PJRT init so the .so's client exists
        if device_ids:
            ids = (ctypes.c_int64 * len(device_ids))(*device_ids)
            rc = lib.axon_start_nrt_profile(ids, len(device_ids))
        else:
            rc = lib.axon_start_nrt_profile(None, 0)
        if rc != 0:
            raise RuntimeError(f"axon_start_nrt_profile rc={rc}")
        try:
            yield
        finally:
            n = lib.axon_stop_nrt_profile(str(output_dir).encode())
            print(f"ntff profile: {n} file(s) -> {output_dir}")

    import antenv

    mod = types.ModuleType("antenv.axon_hooks")
    mod.get_axon_ntff_profile_hook = lambda: _hook
    mod.set_axon_ntff_profile_hook = lambda h: None
    sys.modules["antenv.axon_hooks"] = mod
    antenv.axon_hooks = mod
    return True


def _build_bass(n_main: int, ov: bool) -> bass.Bass:
    """SPMD kernel for one core.

    Main tiles: y[mt] = xt[mt].T @ w for mt in 0..n_main-1 (full K=2048).
    Final tile: ov=True  -> y2 = xt2.T @ w2 with K=1024 (overflow half).
                ov=False -> the last main m-tile (mt = n_main, full K),
                            run nt-major for tail pipelining.

    xt: [MTx*128, KT*128] where row mt*128+p, col kt*128+t holds
        x_token[mt*128+t, kt*128+p] (per-m-tile transposed blocks).
    w:  [H, H] row-major.  y: [MTx*128, H] bf16.
    xt2: [128, 1024], w2: [1024, H], y2: [128, H] (ov mode only).
    """
    assert n_main >= 3
    f32 = mybir.dt.float32
    bf16 = mybir.dt.bfloat16
    MTx = n_main if ov else n_main + 1  # m-tiles in xt/y
    KTF = (H // 2 if ov else H) // P  # k-tiles of the final tile

    nc = bass.Bass()
    xt = nc.dram_tensor("xt", [MTx * P, KT * P], bf16, kind="ExternalInput")
    w = nc.dram_tensor("w", [H, H], bf16, kind="ExternalInput")
    y = nc.dram_tensor("y", [MTx * P, H], bf16, kind="ExternalOutput")
    if ov:
        xt2 = nc.dram_tensor("xt2", [P, KTF * P], bf16, kind="ExternalInput")
        w2 = nc.dram_tensor("w2", [KTF * P, H], bf16, kind="ExternalInput")
        y2 = nc.dram_tensor("y2", [P, H], bf16, kind="ExternalOutput")

    with (
        nc.sbuf_tensor("w_sb", [P, KT, H], bf16) as w_sb,
        nc.sbuf_tensor("x_sb", [P, n_main, H], bf16) as x_sb,
        nc.sbuf_tensor("y_sb", [P, n_main, H], bf16) as y_sb,
        nc.sbuf_tensor("xf_sb", [P, KTF * P], bf16) as xf_sb,
        nc.sbuf_tensor("yf_sb", [P, H], bf16) as yf_sb,
        nc.sbuf_tensor(
            "wf_sb", [P, KTF if ov else 1, H if ov else 2], bf16
        ) as wf_alloc,
        nc.sbuf_tensor("warm", [P, WARM_N], bf16) as warm,
        nc.psum_tensor("ps0", [P, H], f32) as ps0,
        nc.psum_tensor("ps1", [P, H], f32) as ps1,
        nc.semaphore("sPE") as sPE,
        nc.semaphore("sCopy") as sCopy,
        nc.semaphore("sWarm") as sWarm,
        nc.semaphore("sXf") as sXf,
        nc.semaphore("sCLv") as sCLv,
        nc.semaphore("sCLg") as sCLg,
        nc.semaphore("sYsync") as sYsync,
        nc.semaphore("sYscal") as sYscal,
        nc.semaphore("sW0b") as sW0b,
        nc.semaphore("sX0b") as sX0b,
        nc.Block() as block,
    ):
        psums = [ps0, ps1]
        sW = [nc.semaphore(f"sW{kt}").__enter__() for kt in range(KT)]
        sX = [nc.semaphore(f"sX{mt}").__enter__() for mt in range(n_main)]
        sY = [nc.semaphore(f"sY{mt}").__enter__() for mt in range(n_main)]
        ps_f = psums[n_main % 2]
        if ov:
            wf_sb = wf_alloc
            sWf = [nc.semaphore(f"sWf{kt}").__enter__() for kt in range(KTF)]
        else:
            wf_sb, sWf = w_sb, sW  # final tile reuses resident W

        USE_GPSIMD_X = os.environ.get("MOE_GPSIMD_X", "0") == "1"
        HHG = H // 2

        if USE_GPSIMD_X:

            @block.gpsimd
            def _(gp):
                # SWDGE: x tiles 0,1 in parallel with the W HWDGE streams.
                gp.dma_start(x_sb[:, 0, 0:HHG], xt[0:P, 0:HHG]).then_inc(
                    sX[0], 16
                )
                gp.dma_start(x_sb[:, 0, HHG:H], xt[0:P, HHG:H]).then_inc(
                    sX0b, 16
                )
                gp.dma_start(x_sb[:, 1, :], xt[P : 2 * P, :]).then_inc(sX[1], 16)
                if ov:
                    gp.dma_start(xf_sb[:, :], xt2[:, :]).then_inc(sXf, 16)
                else:
                    gp.dma_start(
                        xf_sb[:, :], xt[n_main * P : (n_main + 1) * P, :]
                    ).then_inc(sXf, 16)


        yf_dst = y2 if ov else y
        r0 = 0 if ov else n_main * P

        HH = H // 2

        @block.sync
        def _(sync):
            # W0 first-half + x0 first-half first (minimal bytes before the
            # first real matmul), then even W k-tiles, x tiles 2.., even w2
            # tiles, final stores 0,1,3
            sync.dma_start(w_sb[:, 0, 0:HH], w[0:P, 0:HH]).then_inc(sW[0], 16)
            if not USE_GPSIMD_X:
                sync.dma_start(x_sb[:, 0, 0:HH], xt[0:P, 0:HH]).then_inc(
                    sX[0], 16
                )
                sync.dma_start(x_sb[:, 1, :], xt[P : 2 * P, :]).then_inc(
                    sX[1], 16
                )
            for kt in range(2, KT, 2):
                sync.dma_start(
                    w_sb[:, kt, :], w[kt * P : (kt + 1) * P, :]
                ).then_inc(sW[kt], 16)
            for mt in range(2, n_main):
                sync.dma_start(
                    x_sb[:, mt, :], xt[mt * P : (mt + 1) * P, :]
                ).then_inc(sX[mt], 16)
            if not USE_GPSIMD_X:
                if ov:
                    sync.dma_start(xf_sb[:, :], xt2[:, :]).then_inc(sXf, 16)
                else:
                    sync.dma_start(
                        xf_sb[:, :], xt[n_main * P : (n_main + 1) * P, :]
                    ).then_inc(sXf, 16)
            if ov:
                for kt in range(0, KTF, 2):
                    sync.dma_start(
                        wf_sb[:, kt, :], w2[kt * P : (kt + 1) * P, :]
                    ).then_inc(sWf[kt], 16)
            for i, nt in enumerate((0, 1, 3)):
                sync.wait_ge(sCLv, i + 1)
                sync.dma_start(
                    yf_dst[r0 : r0 + P, nt * N_FREE : (nt + 1) * N_FREE],
                    yf_sb[:, nt * N_FREE : (nt + 1) * N_FREE],
                ).then_inc(sYsync, 16)
            sync.wait_ge(sYsync, 48)

        @block.scalar
        def _(scalar):
            # W0 second-half, W1, x1, x0 second-half, odd W k-tiles, odd w2
            # tiles, main stores, final store 2
            scalar.dma_start(w_sb[:, 0, HH:H], w[0:P, HH:H]).then_inc(
                sW0b, 16
            )
            scalar.dma_start(w_sb[:, 1, :], w[P : 2 * P, :]).then_inc(
                sW[1], 16
            )
            if not USE_GPSIMD_X:
                scalar.dma_start(x_sb[:, 0, HH:H], xt[0:P, HH:H]).then_inc(
                    sX0b, 16
                )
            for kt in range(3, KT, 2):
                scalar.dma_start(
                    w_sb[:, kt, :], w[kt * P : (kt + 1) * P, :]
                ).then_inc(sW[kt], 16)
            if ov:
                for kt in range(1, KTF, 2):
                    scalar.dma_start(
                        wf_sb[:, kt, :], w2[kt * P : (kt + 1) * P, :]
                    ).then_inc(sWf[kt], 16)
            for mt in range(n_main):
                scalar.wait_ge(sCopy, mt + 1)
                scalar.dma_start(
                    y[mt * P : (mt + 1) * P, :], y_sb[:, mt, :]
                ).then_inc(sY[mt], 16)
            for i, nt in enumerate((2,)):
                # ACT casts its own slice then stores it; the sem wait
                # orders the DMA behind the copy's SBUF writes (same-engine
                # issue does NOT imply write completion).
                scalar.wait_ge(sPE, n_main + nt + 1)
                scalar.copy(
                    yf_sb[:, nt * N_FREE : (nt + 1) * N_FREE],
                    ps_f[:, nt * N_FREE : (nt + 1) * N_FREE],
                ).then_inc(sCLg, 1)
                scalar.wait_ge(sCLg, i + 1)
                scalar.dma_start(
                    yf_dst[r0 : r0 + P, nt * N_FREE : (nt + 1) * N_FREE],
                    yf_sb[:, nt * N_FREE : (nt + 1) * N_FREE],
                ).then_inc(sYscal, 16)
            for mt in range(n_main):
                scalar.wait_ge(sY[mt], 16)
            scalar.wait_ge(sYscal, 16)

        @block.tensor
        def _(tensor):
            def mm(psum, mt, kt, nt, start, stop):
                return tensor.matmul(
                    psum[:, nt * N_FREE : (nt + 1) * N_FREE],
                    x_sb[:, mt, kt * P : (kt + 1) * P],
                    w_sb[:, kt, nt * N_FREE : (nt + 1) * N_FREE],
                    start=start,
                    stop=stop,
                    skip_group_check=True,
                )

            def mm4(psum, mt, kt):
                for nt in range(NT):
                    m = mm(psum, mt, kt, nt, kt == 0, kt == KT - 1)
                return m

            # Pre-warm the PE (HAM un-throttles after ~3.4us of activity)
            # on scratch data while the first DMAs land.
            tensor.wait_ge(sWarm, 1)
            for _ in range(N_WARM):
                tensor.matmul(
                    ps0[:, 0:WARM_N],
                    warm[:, 0:P],
                    warm[:, :],
                    start=True,
                    stop=True,
                    skip_group_check=True,
                )

            # Phase 1: m-tiles 0,1 k-major chasing the W DMA streams,
            # m-tile 1 lagging LAG k-tiles.
            tensor.wait_ge(sX[0], 16)
            for kt in range(KT):
                tensor.wait_ge(sW[kt], 16)
                if kt == 0:
                    # W0/x0 second halves stream in behind the first halves
                    for nt in (0, 1):
                        mm(ps0, 0, 0, nt, True, False)
                    tensor.wait_ge(sW0b, 16)
                    for nt in (2, 3):
                        m = mm(ps0, 0, 0, nt, True, False)
                else:
                    if kt == KT // 2:
                        tensor.wait_ge(sX0b, 16)
                    m = mm4(ps0, 0, kt)
                if kt == KT - 1:
                    m.then_inc(sPE, 1)
                if kt == LAG:
                    tensor.wait_ge(sX[1], 16)
                if kt >= LAG:
                    mm4(ps1, 1, kt - LAG)
            for kt in range(KT - LAG, KT):
                m = mm4(ps1, 1, kt)
            m.then_inc(sPE, 1)
            # Phase 2: W resident; stream the remaining m-tiles.
            for mt in range(2, n_main):
                tensor.wait_ge(sX[mt], 16)
                tensor.wait_ge(sCopy, mt - 1)  # psum slot free
                for kt in range(KT):
                    m = mm4(psums[mt % 2], mt, kt)
                m.then_inc(sPE, 1)
            # Final tile: nt-major so each col-slice finishes early and
            # its cast+store pipelines behind the remaining matmuls.
            tensor.wait_ge(sXf, 16)
            tensor.wait_ge(sCopy, n_main - 1)
            for nt in range(NT):
                for kt in range(KTF):
                    if nt == 0:
                        tensor.wait_ge(sWf[kt], 16)
                    m = tensor.matmul(
                        ps_f[:, nt * N_FREE : (nt + 1) * N_FREE],
                        xf_sb[:, kt * P : (kt + 1) * P],
                        wf_sb[:, kt, nt * N_FREE : (nt + 1) * N_FREE],
                        start=(kt == 0),
                        stop=(kt == KTF - 1),
                        skip_group_check=True,
                    )
                m.then_inc(sPE, 1)

        @block.vector
        def _(vector):
            vector.memset(warm[:, :], 0.25).then_inc(sWarm, 1)
            for mt in range(n_main):
                vector.wait_ge(sPE, mt + 1)
                vector.tensor_copy(
                    y_sb[:, mt, :], psums[mt % 2][:, :]
                ).then_inc(sCopy, 1)
            for nt in (0, 1, 3):
                vector.wait_ge(sPE, n_main + nt + 1)
                vector.tensor_copy(
                    yf_sb[:, nt * N_FREE : (nt + 1) * N_FREE],
                    ps_f[:, nt * N_FREE : (nt + 1) * N_FREE],
                ).then_inc(sCLv, 1)

    return nc


def _route(x, Wg):
    """Host gating: returns token indices per expert and top-1 probs."""
    xf = np.ascontiguousarray(x.reshape(-1, H))
    logits = xf @ Wg  # [T, E] fp32 (min top1-top2 gap ~1e-4)
    idx = logits.argmax(-1)
    m = logits.max(-1, keepdims=True)
    ex = np.exp(logits - m)
    p = (ex[np.arange(len(idx)), idx] / ex.sum(-1)).astype(np.float32)
    return xf, idx, p


def _pack_tiles(xs: np.ndarray, n_tiles: int, k: int) -> np.ndarray:
    """[n_tok, k] fp32 -> [n_tiles*128, k] bf16 per-m-tile transposed tiles.

    Row mt*128+p, col kt*128+t  <-  xs[mt*128+t, kt*128+p].
    """
    n = xs.shape[0]
    kt = k // P
    out = np.zeros((n_tiles * P, k), dtype=BF16)
    for mt in range(n_tiles):
        t0, t1 = mt * P, min((mt + 1) * P, n)
        if t0 >= t1:
            break
        blk = xs[t0:t1].astype(BF16)  # [tc, k]
        tc = t1 - t0
        dst = out[mt * P : (mt + 1) * P].reshape(P, kt, P)  # [p, kt, t]
        dst[:, :, :tc] = blk.reshape(tc, kt, P).transpose(2, 1, 0)
    return out


def _run(inputs, trace=False):
    x = np.asarray(inputs["x"], dtype=np.float32)
    Wg = np.asarray(inputs["Wg"], dtype=np.float32)
    W = np.asarray(inputs["W"], dtype=np.float32)
    b = np.asarray(inputs["b"], dtype=np.float32)

    if trace:
        trace = _ensure_ntff_hook()

    xf, idx, p = _route(x, Wg)
    T = xf.shape[0]

    toks = [np.nonzero(idx == e)[0] for e in range(E)]
    counts = np.array([len(t) for t in toks])

    # Overflow pieces: per-expert token chunks beyond CAP, each <= 128.
    pieces = []
    for e in range(E):
        o = toks[e][CAP:]
        for i in range(0, len(o), P):
            pieces.append((e, o[i : i + P]))

    ov = 0 < len(pieces) <= N_CORES // 2 and counts.max() > CAP
    if ov:
        n_main = CAP // P
        key = ("OV", n_main)
    else:
        n_main = max(3, int(-(-counts.max() // P)) - 1)
        key = ("A", n_main)
    if key not in _COMPILED:
        _COMPILED[key] = _build_bass(n_main, ov)
    nc = _COMPILED[key]

    KH = H // 2
    in_maps = []
    for c in range(N_CORES):
        e = c
        te = toks[e][: CAP if ov else None]
        xs = xf[te] * p[te, None]  # fold gate prob into activations
        m = {
            "xt": _pack_tiles(xs, n_main if ov else n_main + 1, H),
            "w": W[e].astype(BF16),
        }
        if ov:
            g, h = c // 2, c % 2
            if g < len(pieces):
                e2, t2 = pieces[g]
                xs2 = (xf[t2] * p[t2, None])[:, h * KH : (h + 1) * KH]
                m["xt2"] = _pack_tiles(xs2, 1, KH)
                m["w2"] = W[e2][h * KH : (h + 1) * KH].astype(BF16)
            else:
                m["xt2"] = np.zeros((P, KH), dtype=BF16)
                m["w2"] = np.zeros((KH, H), dtype=BF16)
        in_maps.append(m)

    res = run_bass_kernel_spmd(
        nc,
        in_maps,
        core_ids=list(range(N_CORES)),
        trace=trace,
        trace_cores=list(range(N_CORES)) if trace else None,
    )

    out = np.empty((T, H), dtype=np.float32)
    for e in range(E):
        te = toks[e][: CAP if ov else None]
        ye = res.results[e]["y"][: len(te)].astype(np.float32)
        if np.any(b[e]):
            ye = ye + p[te, None] * b[e]
        out[te] = ye
    if ov:
        for g in range(len(pieces)):
            e2, t2 = pieces[g]
            ye = (
                res.results[2 * g]["y2"][: len(t2)].astype(np.float32)
                + res.results[2 * g + 1]["y2"][: len(t2)].astype(np.float32)
            )
            if np.any(b[e2]):
                ye = ye + p[t2, None] * b[e2]
            out[t2] = ye
    return out.reshape(B, S, H), res


def kernel(**inputs) -> np.ndarray:
    out, _ = _run(inputs, trace=os.environ.get("MOE_TRACE", "0") == "1")
    return out


def run_traced(inputs):
    """For test.py: returns (output, BassKernelResults with exec_time_ns)."""
    return _run(inputs, trace=True)


# revision 35
# speedup vs baseline: 1.0181x; 1.0090x over previous
"""Top-1 MoE block (B=4, S=2048, H=2048, E=8) for 8 Trainium2 NeuronCores.

Strategy (expert-parallel, host-mediated dispatch):
  - Host computes the tiny gating network (x @ Wg -> softmax -> argmax),
    0.4% of total FLOPs, and the token permutation per expert.
  - Token block for expert e (prob-scaled, cast to bf16, tiled) plus
    W[e] (bf16) goes to core e.  Each core runs a dense matmul in bf16
    (full PE rate, half the HBM traffic of fp32).
  - Tokens beyond 1024 per expert ("overflow", ~210 of 8192 here) are
    packed into one extra half-contraction tile per core: a core pair
    (2g, 2g+1) computes K-halves [0:1024) / [1024:2048) of overflow
    group g; the host sums the two partial outputs.  This keeps every
    core at 8 full m-tiles + 1 half-K tile instead of 9 full tiles.
  - Host upcasts per-expert outputs, scatters back to token order.

Schedule per core (all numbers warm-clock):
  - The first 256 tokens (m-tiles 0,1) are packed as four quarter-K
    blocks (xq) interleaved into the two HWDGE queues with the W
    k-tiles, ordered so PE consumption (~1.7us per 0.5MB k-tile at
    8 matmuls/tile) tracks the ~200GB/s-per-queue delivery curve.
  - Tensor engine pre-warms the PE (HAM un-throttle) with junk matmuls
    until the first blocks land, then runs k-major over m-tiles {0,1},
    then streams the remaining m-tiles with W resident.
  - m-tile 0 is copied out of PSUM in four nt-slices and m-tile 2 runs
    nt-major gated per-slice, so the PSUM handoff has no bubble.
  - The final (overflow) tile runs nt-major so its casts+stores
    pipeline behind the last matmuls (vector nt0/1/3, ACT nt2).
"""

import os

import numpy as np
import ml_dtypes

import concourse.bass as bass
from concourse import mybir
from concourse.bass_utils import run_bass_kernel_spmd

BF16 = ml_dtypes.bfloat16

B, S, H, E = 4, 2048, 2048, 8
P = 128
KT = H // P  # 16 k tiles
N_FREE = 512  # matmul moving free dim / PSUM bank width (fp32)
NT = H // N_FREE  # 4 n tiles
N_CORES = 8
N_WARM = 23  # pre-warm matmuls, N=256 (~213ns cold / ~110ns warm each)
WARM_N = 256
CAP = 1024  # main-tile token capacity per core in overflow mode

_COMPILED = {}


def _ensure_ntff_hook() -> bool:
    """Register antenv.axon_hooks with a ctypes NTFF hook if the image lacks it."""
    import contextlib
    import ctypes
    import sys
    import types

    try:
        from antenv.axon_hooks import get_axon_ntff_profile_hook  # noqa: F401

        return True
    except ImportError:
        pass

    so_path = "/opt/axon/libaxon_pjrt.so"
    if not os.path.exists(so_path):
        return False
    lib = ctypes.CDLL(so_path)
    if not hasattr(lib, "axon_start_nrt_profile"):
        return False
    lib.axon_start_nrt_profile.argtypes = [
        ctypes.POINTER(ctypes.c_int64),
        ctypes.c_size_t,
    ]
    lib.axon_start_nrt_profile.restype = ctypes.c_int64
    lib.axon_stop_nrt_profile.argtypes = [ctypes.c_char_p]
    lib.axon_stop_nrt_profile.restype = ctypes.c_int64

    @contextlib.contextmanager
    def _hook(output_dir, device_ids):
        import jax

        jax.devices()  # force PJRT init so the .so's client exists
        if device_ids:
            ids = (ctypes.c_int64 * len(device_ids))(*device_ids)
            rc = lib.axon_start_nrt_profile(ids, len(device_ids))
        else:
            rc = lib.axon_start_nrt_profile(None, 0)
        if rc != 0:
            raise RuntimeError(f"axon_start_nrt_profile rc={rc}")
        try:
            yield
        finally:
            n = lib.axon_stop_nrt_profile(str(output_dir).encode())
            print(f"ntff profile: {n} file(s) -> {output_dir}")

    import antenv

    mod = types.ModuleType("antenv.axon_hooks")
    mod.get_axon_ntff_profile_hook = lambda: _hook
    mod.set_axon_ntff_profile_hook = lambda h: None
    sys.modules["antenv.axon_hooks"] = mod
    antenv.axon_hooks = mod
    return True


def _build_bass(n_main: int, ov: bool) -> bass.Bass:
    """SPMD kernel for one core.

    Main tiles: y[mt] = x[mt].T @ w for mt in 0..n_main-1 (full K=2048).
    Final tile: ov=True  -> y2 = xt2.T @ w2 with K=1024 (overflow half).
                ov=False -> one more main m-tile (full K), nt-major.

    xq:  [512, 1024] four quarter-K blocks of m-tiles 0,1:
         row q*128+p, col kl*256+t = x_token[t][(4q+kl)*128+p], t in
         0..255 spanning both m-tiles.
    xt:  [(MTx-2)*128, 2048] per-m-tile transposed blocks for tiles 2..:
         row (mt-2)*128+p, col kt*128+t = x_token[mt*128+t][kt*128+p].
    w:   [H, H] row-major.  y: [MTx*128, H] bf16.
    xt2: [128, 1024], w2: [1024, H], y2: [128, H] (ov mode only).
    """
    assert n_main >= 4
    f32 = mybir.dt.float32
    bf16 = mybir.dt.bfloat16
    MTx = n_main if ov else n_main + 1  # m-tiles in y
    KTF = (H // 2 if ov else H) // P  # k-tiles of the final tile
    HH = H // 2

    nc = bass.Bass()
    xq = nc.dram_tensor("xq", [4 * P, 4 * 2 * P], bf16, kind="ExternalInput")
    xt = nc.dram_tensor(
        "xt", [(MTx - 2) * P, KT * P], bf16, kind="ExternalInput"
    )
    w = nc.dram_tensor("w", [H, H], bf16, kind="ExternalInput")
    y = nc.dram_tensor("y", [MTx * P, H], bf16, kind="ExternalOutput")
    if ov:
        xt2 = nc.dram_tensor("xt2", [P, KTF * P], bf16, kind="ExternalInput")
        w2 = nc.dram_tensor("w2", [KTF * P, H], bf16, kind="ExternalInput")
        y2 = nc.dram_tensor("y2", [P, H], bf16, kind="ExternalOutput")

    with (
        nc.sbuf_tensor("w_sb", [P, KT, H], bf16) as w_sb,
        nc.sbuf_tensor("x01_sb", [P, KT, 2 * P], bf16) as x01_sb,
        nc.sbuf_tensor("x_sb", [P, n_main - 2, H], bf16) as x_sb,
        nc.sbuf_tensor("y_sb", [P, n_main, H], bf16) as y_sb,
        nc.sbuf_tensor("xf_sb", [P, KTF * P], bf16) as xf_sb,
        nc.sbuf_tensor("yf_sb", [P, H], bf16) as yf_sb,
        nc.sbuf_tensor(
            "wf_sb", [P, KTF if ov else 1, H if ov else 2], bf16
        ) as wf_alloc,
        nc.sbuf_tensor("warm", [P, WARM_N], bf16) as warm,
        nc.psum_tensor("ps0", [P, H], f32) as ps0,
        nc.psum_tensor("ps1", [P, H], f32) as ps1,
        nc.semaphore("sPE") as sPE,
        nc.semaphore("sCopy") as sCopy,
        nc.semaphore("sWarm") as sWarm,
        nc.semaphore("sXf") as sXf,
        nc.semaphore("sCLv") as sCLv,
        nc.semaphore("sCLg") as sCLg,
        nc.semaphore("sYsync") as sYsync,
        nc.semaphore("sYscal") as sYscal,
        nc.semaphore("sW0b") as sW0b,
        nc.Block() as block,
    ):
        psums = [ps0, ps1]
        sW = [nc.semaphore(f"sW{kt}").__enter__() for kt in range(KT)]
        sXQ = [nc.semaphore(f"sXQ{q}").__enter__() for q in range(4)]
        sX = [nc.semaphore(f"sX{mt}").__enter__() for mt in range(2, n_main)]
        sY = [nc.semaphore(f"sY{mt}").__enter__() for mt in range(n_main)]
        ps_f = psums[n_main % 2]
        if ov:
            wf_sb = wf_alloc
            sWf = [nc.semaphore(f"sWf{kt}").__enter__() for kt in range(KTF)]
        else:
            wf_sb, sWf = w_sb, sW  # final tile reuses resident W

        def w_dma(eng, kt):
            eng.dma_start(
                w_sb[:, kt, :], w[kt * P : (kt + 1) * P, :]
            ).then_inc(sW[kt], 16)

        def xq_dma(eng, q):
            eng.dma_start(
                x01_sb[:, 4 * q : 4 * (q + 1), :], xq[q * P : (q + 1) * P, :]
            ).then_inc(sXQ[q], 16)

        def x_dma(eng, mt):
            eng.dma_start(
                x_sb[:, mt - 2, :], xt[(mt - 2) * P : (mt - 1) * P, :]
            ).then_inc(sX[mt - 2], 16)

        yf_dst = y2 if ov else y
        r0 = 0 if ov else n_main * P

        @block.sync
        def _(sync):
            # interleaved head: xq quarters + W0a + even W k-tiles, then
            # late x tiles, even w2 tiles, final stores 0,1,3
            xq_dma(sync, 0)
            sync.dma_start(w_sb[:, 0, 0:HH], w[0:P, 0:HH]).then_inc(sW[0], 16)
            xq_dma(sync, 1)
            w_dma(sync, 2)
            xq_dma(sync, 3)
            for kt in range(4, KT, 2):
                w_dma(sync, kt)
            for mt in range(4, n_main):
                x_dma(sync, mt)
            if ov:
                for kt in range(0, KTF, 2):
                    sync.dma_start(
                        wf_sb[:, kt, :], w2[kt * P : (kt + 1) * P, :]
                    ).then_inc(sWf[kt], 16)
            for i, nt in enumerate((0, 1, 3)):
                sync.wait_ge(sCLv, i + 1)
                sync.dma_start(
                    yf_dst[r0 : r0 + P, nt * N_FREE : (nt + 1) * N_FREE],
                    yf_sb[:, nt * N_FREE : (nt + 1) * N_FREE],
                ).then_inc(sYsync, 16)
            sync.wait_ge(sYsync, 48)

        @block.scalar
        def _(scalar):
            # W0b, W1, xq2, odd W k-tiles, x2/x3, xf, odd w2 tiles,
            # main stores, final store 2
            scalar.dma_start(w_sb[:, 0, HH:H], w[0:P, HH:H]).then_inc(
                sW0b, 16
            )
            w_dma(scalar, 1)
            xq_dma(scalar, 2)
            for kt in range(3, KT, 2):
                w_dma(scalar, kt)
            x_dma(scalar, 2)
            x_dma(scalar, 3)
            if ov:
                scalar.dma_start(xf_sb[:, :], xt2[:, :]).then_inc(sXf, 16)
                for kt in range(1, KTF, 2):
                    scalar.dma_start(
                        wf_sb[:, kt, :], w2[kt * P : (kt + 1) * P, :]
                    ).then_inc(sWf[kt], 16)
            else:
                scalar.dma_start(
                    xf_sb[:, :], xt[(n_main - 2) * P : (n_main - 1) * P, :]
                ).then_inc(sXf, 16)
            for mt in range(n_main):
                scalar.wait_ge(sCopy, mt + 4 if mt else 4)
                scalar.dma_start(
                    y[mt * P : (mt + 1) * P, :], y_sb[:, mt, :]
                ).then_inc(sY[mt], 16)
            # ACT casts its own final slice then stores it; the sem wait
            # orders the DMA behind the copy's SBUF writes (same-engine
            # issue does NOT imply write completion).
            scalar.wait_ge(sPE, n_main + 6)
            scalar.copy(
                yf_sb[:, 2 * N_FREE : 3 * N_FREE],
                ps_f[:, 2 * N_FREE : 3 * N_FREE],
            ).then_inc(sCLg, 1)
            scalar.wait_ge(sCLg, 1)
            scalar.dma_start(
                yf_dst[r0 : r0 + P, 2 * N_FREE : 3 * N_FREE],
                yf_sb[:, 2 * N_FREE : 3 * N_FREE],
            ).then_inc(sYscal, 16)
            for mt in range(n_main):
                scalar.wait_ge(sY[mt], 16)
            scalar.wait_ge(sYscal, 16)

        @block.tensor
        def _(tensor):
            def mm01(psum, mt, kt, nt, start, stop):
                return tensor.matmul(
                    psum[:, nt * N_FREE : (nt + 1) * N_FREE],
                    x01_sb[:, kt, mt * P : (mt + 1) * P],
                    w_sb[:, kt, nt * N_FREE : (nt + 1) * N_FREE],
                    start=start,
                    stop=stop,
                    skip_group_check=True,
                )

            def mm(psum, mt, kt, nt, start, stop):
                return tensor.matmul(
                    psum[:, nt * N_FREE : (nt + 1) * N_FREE],
                    x_sb[:, mt - 2, kt * P : (kt + 1) * P],
                    w_sb[:, kt, nt * N_FREE : (nt + 1) * N_FREE],
                    start=start,
                    stop=stop,
                    skip_group_check=True,
                )

            # Pre-warm the PE (HAM un-throttles after ~3.4us of activity)
            # on scratch data while the first DMAs land.
            tensor.wait_ge(sWarm, 1)
            for _ in range(N_WARM):
                tensor.matmul(
                    ps0[:, 0:WARM_N],
                    warm[:, 0:P],
                    warm[:, :],
                    start=True,
                    stop=True,
                    skip_group_check=True,
                )

            # Phase 1: m-tiles 0,1 k-major chasing the W/xq DMA streams.
            for kt in range(KT):
                if kt % 4 == 0:
                    tensor.wait_ge(sXQ[kt // 4], 16)
                tensor.wait_ge(sW[kt], 16)
                if kt == 0:
                    tensor.wait_ge(sW0b, 16)
                last = kt == KT - 1
                for nt in range(NT):
                    m = mm01(ps0, 0, kt, nt, kt == 0, last)
                    if last:
                        m.then_inc(sPE, 1)  # per-nt: mt0 copy pipelines
                for nt in range(NT):
                    m = mm01(ps1, 1, kt, nt, kt == 0, last)
                if last:
                    m.then_inc(sPE, 1)
            # Phase 2 head: m-tile 2 nt-major, gated on m-tile 0's
            # per-slice PSUM copies (no handoff bubble).
            tensor.wait_ge(sX[0], 16)
            for nt in range(NT):
                tensor.wait_ge(sCopy, nt + 1)
                for kt in range(KT):
                    m = mm(ps0, 2, kt, nt, kt == 0, kt == KT - 1)
            m.then_inc(sPE, 1)
            # Phase 2: W resident; stream the remaining m-tiles.
            for mt in range(3, n_main):
                tensor.wait_ge(sX[mt - 2], 16)
                tensor.wait_ge(sCopy, mt + 2)  # psum slot free
                for kt in range(KT):
                    for nt in range(NT):
                        m = mm(psums[mt % 2], mt, kt, nt, kt == 0, kt == KT - 1)
                m.then_inc(sPE, 1)
            # Final tile: nt-major so each col-slice finishes early and
            # its cast+store pipelines behind the remaining matmuls.
            tensor.wait_ge(sXf, 16)
            tensor.wait_ge(sCopy, n_main + 2)
            for nt in range(NT):
                for kt in range(KTF):
                    if nt == 0:
                        tensor.wait_ge(sWf[kt], 16)
                    m = tensor.matmul(
                        ps_f[:, nt * N_FREE : (nt + 1) * N_FREE],
                        xf_sb[:, kt * P : (kt + 1) * P],
                        wf_sb[:, kt, nt * N_FREE : (nt + 1) * N_FREE],
                        start=(kt == 0),
                        stop=(kt == KTF - 1),
                        skip_group_check=True,
                    )
                m.then_inc(sPE, 1)

        @block.vector
        def _(vector):
            vector.memset(warm[:, :], 0.25).then_inc(sWarm, 1)
            # m-tile 0 in nt-slices (pipelines with mt1 kt15 + mt2 nt-major)
            for nt in range(NT):
                vector.wait_ge(sPE, nt + 1)
                vector.tensor_copy(
                    y_sb[:, 0, nt * N_FREE : (nt + 1) * N_FREE],
                    ps0[:, nt * N_FREE : (nt + 1) * N_FREE],
                ).then_inc(sCopy, 1)
            for mt in range(1, n_main):
                vector.wait_ge(sPE, mt + 4)
                vector.tensor_copy(
                    y_sb[:, mt, :], psums[mt % 2][:, :]
                ).then_inc(sCopy, 1)
            for i, nt in enumerate((0, 1, 3)):
                vector.wait_ge(sPE, n_main + 4 + nt)
                vector.tensor_copy(
                    yf_sb[:, nt * N_FREE : (nt + 1) * N_FREE],
                    ps_f[:, nt * N_FREE : (nt + 1) * N_FREE],
                ).then_inc(sCLv, 1)

    return nc


def _route(x, Wg):
    """Host gating: returns token indices per expert and top-1 probs."""
    xf = np.ascontiguousarray(x.reshape(-1, H))
    logits = xf @ Wg  # [T, E] fp32 (min top1-top2 gap ~1e-4)
    idx = logits.argmax(-1)
    m = logits.max(-1, keepdims=True)
    ex = np.exp(logits - m)
    p = (ex[np.arange(len(idx)), idx] / ex.sum(-1)).astype(np.float32)
    return xf, idx, p


def _pack_tiles(xs: np.ndarray, n_tiles: int, k: int, t0_tok: int = 0):
    """tokens [t0_tok + mt*128 + t] -> [n_tiles*128, k] bf16 tiles.

    Row mt*128+p, col kt*128+t  <-  xs[t0_tok + mt*128+t, kt*128+p].
    """
    n = xs.shape[0]
    kt = k // P
    out = np.zeros((n_tiles * P, k), dtype=BF16)
    for mt in range(n_tiles):
        t0, t1 = t0_tok + mt * P, min(t0_tok + (mt + 1) * P, n)
        if t0 >= t1:
            break
        blk = xs[t0:t1].astype(BF16)  # [tc, k]
        tc = t1 - t0
        dst = out[mt * P : (mt + 1) * P].reshape(P, kt, P)  # [p, kt, t]
        dst[:, :, :tc] = blk.reshape(tc, kt, P).transpose(2, 1, 0)
    return out


def _pack_xq(xs: np.ndarray) -> np.ndarray:
    """First 256 tokens -> [512, 1024] quarter-K blocks (see _build_bass)."""
    blk = np.zeros((2 * P, H), dtype=BF16)
    n = min(xs.shape[0], 2 * P)
    blk[:n] = xs[:n].astype(BF16)
    a = blk.reshape(2 * P, KT, P).transpose(1, 2, 0)  # [kt, p, t]
    out = np.empty((4 * P, 4 * 2 * P), dtype=BF16)
    for q in range(4):
        out[q * P : (q + 1) * P] = (
            a[4 * q : 4 * (q + 1)].transpose(1, 0, 2).reshape(P, 4 * 2 * P)
        )
    return out


def _run(inputs, trace=False):
    x = np.asarray(inputs["x"], dtype=np.float32)
    Wg = np.asarray(inputs["Wg"], dtype=np.float32)
    W = np.asarray(inputs["W"], dtype=np.float32)
    b = np.asarray(inputs["b"], dtype=np.float32)

    if trace:
        trace = _ensure_ntff_hook()

    xf, idx, p = _route(x, Wg)
    T = xf.shape[0]

    toks = [np.nonzero(idx == e)[0] for e in range(E)]
    counts = np.array([len(t) for t in toks])

    # Overflow pieces: per-expert token chunks beyond CAP, each <= 128.
    pieces = []
    for e in range(E):
        o = toks[e][CAP:]
        for i in range(0, len(o), P):
            pieces.append((e, o[i : i + P]))

    ov = 0 < len(pieces) <= N_CORES // 2
    if ov:
        n_main = CAP // P
        key = ("OV", n_main)
    else:
        n_main = max(4, int(-(-counts.max() // P)) - 1)
        key = ("A", n_main)
    if key not in _COMPILED:
        _COMPILED[key] = _build_bass(n_main, ov)
    nc = _COMPILED[key]

    MTx = n_main if ov else n_main + 1
    KH = H // 2
    in_maps = []
    for c in range(N_CORES):
        e = c
        te = toks[e][: CAP if ov else None]
        xs = xf[te] * p[te, None]  # fold gate prob into activations
        m = {
            "xq": _pack_xq(xs),
            "xt": _pack_tiles(xs, MTx - 2, H, t0_tok=2 * P),
            "w": W[e].astype(BF16),
        }
        if ov:
            g, h = c // 2, c % 2
            if g < len(pieces):
                e2, t2 = pieces[g]
                xs2 = (xf[t2] * p[t2, None])[:, h * KH : (h + 1) * KH]
                m["xt2"] = _pack_tiles(xs2, 1, KH)
                m["w2"] = W[e2][h * KH : (h + 1) * KH].astype(BF16)
            else:
                m["xt2"] = np.zeros((P, KH), dtype=BF16)
                m["w2"] = np.zeros((KH, H), dtype=BF16)
        in_maps.append(m)

    res = run_bass_kernel_spmd(
        nc,
        in_maps,
        core_ids=list(range(N_CORES)),
        trace=trace,
        trace_cores=list(range(N_CORES)) if trace else None,
    )

    out = np.empty((T, H), dtype=np.float32)
    for e in range(E):
        te = toks[e][: CAP if ov else None]
        ye = res.results[e]["y"][: len(te)].astype(np.float32)
        if np.any(b[e]):
            ye = ye + p[te, None] * b[e]
        out[te] = ye
    if ov:
        for g in range(len(pieces)):
            e2, t2 = pieces[g]
            ye = (
                res.results[2 * g]["y2"][: len(t2)].astype(np.float32)
                + res.results[2 * g + 1]["y2"][: len(t2)].astype(np.float32)
            )
            if np.any(b[e2]):
                ye = ye + p[t2, None] * b[e2]
            out[t2] = ye
    return out.reshape(B, S, H), res


def kernel(**inputs) -> np.ndarray:
    out, _ = _run(inputs, trace=os.environ.get("MOE_TRACE", "0") == "1")
    return out


def run_traced(inputs):
    """For test.py: returns (output, BassKernelResults with exec_time_ns)."""
    return _run(inputs, trace=True)


# revision 42
# speedup vs baseline: 1.0288x; 1.0105x over previous
"""Top-1 MoE block (B=4, S=2048, H=2048, E=8) for 8 Trainium2 NeuronCores.

Strategy (expert-parallel, host-mediated dispatch):
  - Host computes the tiny gating network (x @ Wg -> softmax -> argmax),
    0.4% of total FLOPs, and the token permutation per expert.
  - Token block for expert e (prob-scaled, cast to bf16, tiled) plus
    W[e] (bf16) goes to core e.  Each core runs a dense matmul in bf16
    (full PE rate, half the HBM traffic of fp32).
  - Tokens beyond 1024 per expert ("overflow", ~210 of 8192 here) are
    packed into one extra half-contraction tile per core: a core pair
    (2g, 2g+1) computes K-halves [0:1024) / [1024:2048) of overflow
    group g; the host sums the two partial outputs.  This keeps every
    core at 8 full m-tiles + 1 half-K tile instead of 9 full tiles.
  - Host upcasts per-expert outputs, scatters back to token order.

Schedule per core (all numbers warm-clock):
  - The first 256 tokens (m-tiles 0,1) are packed as four quarter-K
    blocks (xq) interleaved into the two HWDGE queues with the W
    k-tiles, ordered so PE consumption (~1.7us per 0.5MB k-tile at
    8 matmuls/tile) tracks the ~200GB/s-per-queue delivery curve.
  - Tensor engine pre-warms the PE (HAM un-throttle) with junk matmuls
    until the first blocks land, then runs k-major over m-tiles {0,1},
    then streams the remaining m-tiles with W resident.
  - m-tile 0 is copied out of PSUM in four nt-slices and m-tile 2 runs
    nt-major gated per-slice, so the PSUM handoff has no bubble.
  - The final (overflow) tile runs nt-major so its casts+stores
    pipeline behind the last matmuls (vector nt0/1/3, ACT nt2).
"""

import os

import numpy as np
import ml_dtypes

import concourse.bass as bass
from concourse import mybir
from concourse.bass_utils import run_bass_kernel_spmd

BF16 = ml_dtypes.bfloat16

B, S, H, E = 4, 2048, 2048, 8
P = 128
KT = H // P  # 16 k tiles
N_FREE = 512  # matmul moving free dim / PSUM bank width (fp32)
NT = H // N_FREE  # 4 n tiles
N_CORES = 8
# final-tile col slices: 3x512 then 2x256 (short tail); index 2 is ACT's
F_SLICES = [(0, 512), (512, 512), (1024, 512), (1536, 512)]
N_WARM = 32  # pre-warm matmuls, N=256 (~213ns cold / ~110ns warm each)
WARM_N = 256
CAP = 1024  # main-tile token capacity per core in overflow mode

_COMPILED = {}


def _ensure_ntff_hook() -> bool:
    """Register antenv.axon_hooks with a ctypes NTFF hook if the image lacks it."""
    import contextlib
    import ctypes
    import sys
    import types

    try:
        from antenv.axon_hooks import get_axon_ntff_profile_hook  # noqa: F401

        return True
    except ImportError:
        pass

    so_path = "/opt/axon/libaxon_pjrt.so"
    if not os.path.exists(so_path):
        return False
    lib = ctypes.CDLL(so_path)
    if not hasattr(lib, "axon_start_nrt_profile"):
        return False
    lib.axon_start_nrt_profile.argtypes = [
        ctypes.POINTER(ctypes.c_int64),
        ctypes.c_size_t,
    ]
    lib.axon_start_nrt_profile.restype = ctypes.c_int64
    lib.axon_stop_nrt_profile.argtypes = [ctypes.c_char_p]
    lib.axon_stop_nrt_profile.restype = ctypes.c_int64

    @contextlib.contextmanager
    def _hook(output_dir, device_ids):
        import jax

        jax.devices()  # force PJRT init so the .so's client exists
        if device_ids:
            ids = (ctypes.c_int64 * len(device_ids))(*device_ids)
            rc = lib.axon_start_nrt_profile(ids, len(device_ids))
        else:
            rc = lib.axon_start_nrt_profile(None, 0)
        if rc != 0:
            raise RuntimeError(f"axon_start_nrt_profile rc={rc}")
        try:
            yield
        finally:
            n = lib.axon_stop_nrt_profile(str(output_dir).encode())
            print(f"ntff profile: {n} file(s) -> {output_dir}")

    import antenv

    mod = types.ModuleType("antenv.axon_hooks")
    mod.get_axon_ntff_profile_hook = lambda: _hook
    mod.set_axon_ntff_profile_hook = lambda h: None
    sys.modules["antenv.axon_hooks"] = mod
    antenv.axon_hooks = mod
    return True


def _build_bass(n_main: int, ov: bool) -> bass.Bass:
    """SPMD kernel for one core.

    Main tiles: y[mt] = x[mt].T @ w for mt in 0..n_main-1 (full K=2048).
    Final tile: ov=True  -> y2 = xt2.T @ w2 with K=1024 (overflow half).
                ov=False -> one more main m-tile (full K), nt-major.

    xq:  [512, 1024] four quarter-K blocks of m-tiles 0,1:
         row q*128+p, col kl*256+t = x_token[t][(4q+kl)*128+p], t in
         0..255 spanning both m-tiles.
    xt:  [(MTx-2)*128, 2048] per-m-tile transposed blocks for tiles 2..:
         row (mt-2)*128+p, col kt*128+t = x_token[mt*128+t][kt*128+p].
    w:   [H, H] row-major.  y: [MTx*128, H] bf16.
    xt2: [128, 1024], w2: [1024, H], y2: [128, H] (ov mode only).
    """
    assert n_main >= 4
    f32 = mybir.dt.float32
    bf16 = mybir.dt.bfloat16
    MTx = n_main if ov else n_main + 1  # m-tiles in y
    KTF = (H // 2 if ov else H) // P  # k-tiles of the final tile
    HH = H // 2

    nc = bass.Bass()
    xq = nc.dram_tensor("xq", [4 * P, 4 * 2 * P], bf16, kind="ExternalInput")
    xt = nc.dram_tensor(
        "xt", [(MTx - 2) * P, KT * P], bf16, kind="ExternalInput"
    )
    w = nc.dram_tensor("w", [H, H], bf16, kind="ExternalInput")
    y = nc.dram_tensor("y", [MTx * P, H], bf16, kind="ExternalOutput")
    if ov:
        xt2 = nc.dram_tensor("xt2", [P, KTF * P], bf16, kind="ExternalInput")
        w2 = nc.dram_tensor("w2", [KTF * P, H], bf16, kind="ExternalInput")
        y2 = nc.dram_tensor("y2", [P, H], bf16, kind="ExternalOutput")

    with (
        nc.sbuf_tensor("w_sb", [P, KT, H], bf16) as w_sb,
        nc.sbuf_tensor("x01_sb", [P, KT, 2 * P], bf16) as x01_sb,
        nc.sbuf_tensor("x_sb", [P, n_main - 2, H], bf16) as x_sb,
        nc.sbuf_tensor("y_sb", [P, n_main, H], bf16) as y_sb,
        nc.sbuf_tensor("xf_sb", [P, KTF * P], bf16) as xf_sb,
        nc.sbuf_tensor("yf_sb", [P, H], bf16) as yf_sb,
        nc.sbuf_tensor(
            "wf_sb", [P, KTF if ov else 1, H if ov else 2], bf16
        ) as wf_alloc,
        nc.sbuf_tensor("warm", [P, WARM_N], bf16) as warm,
        nc.psum_tensor("ps0", [P, H], f32) as ps0,
        nc.psum_tensor("ps1", [P, H], f32) as ps1,
        nc.semaphore("sPE") as sPE,
        nc.semaphore("sCopy") as sCopy,
        nc.semaphore("sWarm") as sWarm,
        nc.semaphore("sXf") as sXf,
        nc.semaphore("sCLv") as sCLv,
        nc.semaphore("sCLg") as sCLg,
        nc.semaphore("sYsync") as sYsync,
        nc.semaphore("sYscal") as sYscal,
        nc.semaphore("sW0b") as sW0b,
        nc.Block() as block,
    ):
        psums = [ps0, ps1]
        sW = [nc.semaphore(f"sW{kt}").__enter__() for kt in range(KT)]
        sXQ = [nc.semaphore(f"sXQ{q}").__enter__() for q in range(4)]
        sX = [nc.semaphore(f"sX{mt}").__enter__() for mt in range(2, n_main)]
        sY = [nc.semaphore(f"sY{mt}").__enter__() for mt in range(n_main)]
        ps_f = psums[n_main % 2]
        if ov:
            wf_sb = wf_alloc
            sWf = [nc.semaphore(f"sWf{kt}").__enter__() for kt in range(KTF)]
        else:
            wf_sb, sWf = w_sb, sW  # final tile reuses resident W

        def w_dma(eng, kt):
            eng.dma_start(
                w_sb[:, kt, :], w[kt * P : (kt + 1) * P, :]
            ).then_inc(sW[kt], 16)

        def xq_dma(eng, q):
            eng.dma_start(
                x01_sb[:, 4 * q : 4 * (q + 1), :], xq[q * P : (q + 1) * P, :]
            ).then_inc(sXQ[q], 16)

        def x_dma(eng, mt):
            eng.dma_start(
                x_sb[:, mt - 2, :], xt[(mt - 2) * P : (mt - 1) * P, :]
            ).then_inc(sX[mt - 2], 16)

        yf_dst = y2 if ov else y
        r0 = 0 if ov else n_main * P

        @block.sync
        def _(sync):
            # interleaved head: xq quarters + W0a + even W k-tiles, then
            # late x tiles, even w2 tiles, final stores 0,1,3
            xq_dma(sync, 0)
            sync.dma_start(w_sb[:, 0, 0:HH], w[0:P, 0:HH]).then_inc(sW[0], 16)
            xq_dma(sync, 1)
            w_dma(sync, 2)
            xq_dma(sync, 3)
            for kt in range(4, KT, 2):
                w_dma(sync, kt)
            for mt in range(4, n_main):
                x_dma(sync, mt)
            if ov:
                for kt in range(0, KTF, 2):
                    sync.dma_start(
                        wf_sb[:, kt, :], w2[kt * P : (kt + 1) * P, :]
                    ).then_inc(sWf[kt], 16)
            n_vs = 0
            for i, (c0, cw) in enumerate(F_SLICES):
                if i == 2:
                    continue  # ACT stores slice 2
                n_vs += 1
                sync.wait_ge(sCLv, n_vs)
                sync.dma_start(
                    yf_dst[r0 : r0 + P, c0 : c0 + cw],
                    yf_sb[:, c0 : c0 + cw],
                ).then_inc(sYsync, 16)
            sync.wait_ge(sYsync, 16 * n_vs)

        @block.scalar
        def _(scalar):
            # W0b, W1, xq2, odd W k-tiles, x2/x3, xf, odd w2 tiles,
            # main stores, final store 2
            scalar.dma_start(w_sb[:, 0, HH:H], w[0:P, HH:H]).then_inc(
                sW0b, 16
            )
            w_dma(scalar, 1)
            xq_dma(scalar, 2)
            for kt in range(3, KT, 2):
                w_dma(scalar, kt)
            x_dma(scalar, 2)
            x_dma(scalar, 3)
            if ov:
                scalar.dma_start(xf_sb[:, :], xt2[:, :]).then_inc(sXf, 16)
                for kt in range(1, KTF, 2):
                    scalar.dma_start(
                        wf_sb[:, kt, :], w2[kt * P : (kt + 1) * P, :]
                    ).then_inc(sWf[kt], 16)
            else:
                scalar.dma_start(
                    xf_sb[:, :], xt[(n_main - 2) * P : (n_main - 1) * P, :]
                ).then_inc(sXf, 16)
            for mt in range(n_main):
                scalar.wait_ge(sCopy, mt + 4 if mt else 4)
                scalar.dma_start(
                    y[mt * P : (mt + 1) * P, :], y_sb[:, mt, :]
                ).then_inc(sY[mt], 16)
            # ACT casts its own final slice then stores it; the sem wait
            # orders the DMA behind the copy's SBUF writes (same-engine
            # issue does NOT imply write completion).
            c0, cw = F_SLICES[2]
            scalar.wait_ge(sPE, n_main + 4 + 2)
            scalar.copy(
                yf_sb[:, c0 : c0 + cw], ps_f[:, c0 : c0 + cw]
            ).then_inc(sCLg, 1)
            scalar.wait_ge(sCLg, 1)
            scalar.dma_start(
                yf_dst[r0 : r0 + P, c0 : c0 + cw], yf_sb[:, c0 : c0 + cw]
            ).then_inc(sYscal, 16)
            for mt in range(n_main):
                scalar.wait_ge(sY[mt], 16)
            scalar.wait_ge(sYscal, 16)

        @block.tensor
        def _(tensor):
            def mm01(psum, mt, kt, nt, start, stop):
                return tensor.matmul(
                    psum[:, nt * N_FREE : (nt + 1) * N_FREE],
                    x01_sb[:, kt, mt * P : (mt + 1) * P],
                    w_sb[:, kt, nt * N_FREE : (nt + 1) * N_FREE],
                    start=start,
                    stop=stop,
                    skip_group_check=True,
                )

            def mm(psum, mt, kt, nt, start, stop):
                return tensor.matmul(
                    psum[:, nt * N_FREE : (nt + 1) * N_FREE],
                    x_sb[:, mt - 2, kt * P : (kt + 1) * P],
                    w_sb[:, kt, nt * N_FREE : (nt + 1) * N_FREE],
                    start=start,
                    stop=stop,
                    skip_group_check=True,
                )

            # Pre-warm the PE (HAM un-throttles after ~3.4us of activity)
            # on scratch data while the first DMAs land.
            tensor.wait_ge(sWarm, 1)
            for _ in range(N_WARM):
                tensor.matmul(
                    ps0[:, 0:WARM_N],
                    warm[:, 0:P],
                    warm[:, :],
                    start=True,
                    stop=True,
                    skip_group_check=True,
                )

            # Phase 1: m-tiles 0,1 k-major chasing the W/xq DMA streams.
            for kt in range(KT):
                if kt % 4 == 0:
                    tensor.wait_ge(sXQ[kt // 4], 16)
                tensor.wait_ge(sW[kt], 16)
                if kt == 0:
                    tensor.wait_ge(sW0b, 16)
                last = kt == KT - 1
                for nt in range(NT):
                    m = mm01(ps0, 0, kt, nt, kt == 0, last)
                    if last:
                        m.then_inc(sPE, 1)  # per-nt: mt0 copy pipelines
                for nt in range(NT):
                    m = mm01(ps1, 1, kt, nt, kt == 0, last)
                if last:
                    m.then_inc(sPE, 1)
            # Phase 2 head: m-tile 2 nt-major, gated on m-tile 0's
            # per-slice PSUM copies (no handoff bubble).
            tensor.wait_ge(sX[0], 16)
            for nt in range(NT):
                tensor.wait_ge(sCopy, nt + 1)
                for kt in range(KT):
                    m = mm(ps0, 2, kt, nt, kt == 0, kt == KT - 1)
            m.then_inc(sPE, 1)
            # Phase 2: W resident; stream the remaining m-tiles.
            for mt in range(3, n_main):
                tensor.wait_ge(sX[mt - 2], 16)
                tensor.wait_ge(sCopy, mt + 2)  # psum slot free
                for kt in range(KT):
                    for nt in range(NT):
                        m = mm(psums[mt % 2], mt, kt, nt, kt == 0, kt == KT - 1)
                m.then_inc(sPE, 1)
            # Final tile: col-slice-major so each slice finishes early and
            # its cast+store pipelines behind the remaining matmuls; the
            # last 512 is split in two 256s to shorten the very tail.
            tensor.wait_ge(sXf, 16)
            tensor.wait_ge(sCopy, n_main + 2)
            for i, (c0, cw) in enumerate(F_SLICES):
                for kt in range(KTF):
                    if i == 0:
                        tensor.wait_ge(sWf[kt], 16)
                    m = tensor.matmul(
                        ps_f[:, c0 : c0 + cw],
                        xf_sb[:, kt * P : (kt + 1) * P],
                        wf_sb[:, kt, c0 : c0 + cw],
                        start=(kt == 0),
                        stop=(kt == KTF - 1),
                        skip_group_check=True,
                    )
                m.then_inc(sPE, 1)

        @block.vector
        def _(vector):
            vector.memset(warm[:, :], 0.25).then_inc(sWarm, 1)
            # m-tile 0 in nt-slices (pipelines with mt1 kt15 + mt2 nt-major)
            for nt in range(NT):
                vector.wait_ge(sPE, nt + 1)
                vector.tensor_copy(
                    y_sb[:, 0, nt * N_FREE : (nt + 1) * N_FREE],
                    ps0[:, nt * N_FREE : (nt + 1) * N_FREE],
                ).then_inc(sCopy, 1)
            for mt in range(1, n_main):
                vector.wait_ge(sPE, mt + 4)
                vector.tensor_copy(
                    y_sb[:, mt, :], psums[mt % 2][:, :]
                ).then_inc(sCopy, 1)
            for i, (c0, cw) in enumerate(F_SLICES):
                if i == 2:
                    continue  # ACT handles slice 2
                vector.wait_ge(sPE, n_main + 4 + i)
                vector.tensor_copy(
                    yf_sb[:, c0 : c0 + cw], ps_f[:, c0 : c0 + cw]
                ).then_inc(sCLv, 1)

    return nc


def _route(x, Wg):
    """Host gating: returns token indices per expert and top-1 probs."""
    xf = np.ascontiguousarray(x.reshape(-1, H))
    logits = xf @ Wg  # [T, E] fp32 (min top1-top2 gap ~1e-4)
    idx = logits.argmax(-1)
    m = logits.max(-1, keepdims=True)
    ex = np.exp(logits - m)
    p = (ex[np.arange(len(idx)), idx] / ex.sum(-1)).astype(np.float32)
    return xf, idx, p


def _pack_tiles(xs: np.ndarray, n_tiles: int, k: int, t0_tok: int = 0):
    """tokens [t0_tok + mt*128 + t] -> [n_tiles*128, k] bf16 tiles.

    Row mt*128+p, col kt*128+t  <-  xs[t0_tok + mt*128+t, kt*128+p].
    """
    n = xs.shape[0]
    kt = k // P
    out = np.zeros((n_tiles * P, k), dtype=BF16)
    for mt in range(n_tiles):
        t0, t1 = t0_tok + mt * P, min(t0_tok + (mt + 1) * P, n)
        if t0 >= t1:
            break
        blk = xs[t0:t1].astype(BF16)  # [tc, k]
        tc = t1 - t0
        dst = out[mt * P : (mt + 1) * P].reshape(P, kt, P)  # [p, kt, t]
        dst[:, :, :tc] = blk.reshape(tc, kt, P).transpose(2, 1, 0)
    return out


def _pack_xq(xs: np.ndarray) -> np.ndarray:
    """First 256 tokens -> [512, 1024] quarter-K blocks (see _build_bass)."""
    blk = np.zeros((2 * P, H), dtype=BF16)
    n = min(xs.shape[0], 2 * P)
    blk[:n] = xs[:n].astype(BF16)
    a = blk.reshape(2 * P, KT, P).transpose(1, 2, 0)  # [kt, p, t]
    out = np.empty((4 * P, 4 * 2 * P), dtype=BF16)
    for q in range(4):
        out[q * P : (q + 1) * P] = (
            a[4 * q : 4 * (q + 1)].transpose(1, 0, 2).reshape(P, 4 * 2 * P)
        )
    return out


def _run(inputs, trace=False):
    x = np.asarray(inputs["x"], dtype=np.float32)
    Wg = np.asarray(inputs["Wg"], dtype=np.float32)
    W = np.asarray(inputs["W"], dtype=np.float32)
    b = np.asarray(inputs["b"], dtype=np.float32)

    if trace:
        trace = _ensure_ntff_hook()

    xf, idx, p = _route(x, Wg)
    T = xf.shape[0]

    toks = [np.nonzero(idx == e)[0] for e in range(E)]
    counts = np.array([len(t) for t in toks])

    # Overflow pieces: per-expert token chunks beyond CAP, each <= 128.
    pieces = []
    for e in range(E):
        o = toks[e][CAP:]
        for i in range(0, len(o), P):
            pieces.append((e, o[i : i + P]))

    ov = 0 < len(pieces) <= N_CORES // 2
    if ov:
        n_main = CAP // P
        key = ("OV", n_main)
    else:
        n_main = max(4, int(-(-counts.max() // P)) - 1)
        key = ("A", n_main)
    if key not in _COMPILED:
        _COMPILED[key] = _build_bass(n_main, ov)
    nc = _COMPILED[key]

    MTx = n_main if ov else n_main + 1
    KH = H // 2
    in_maps = []
    for c in range(N_CORES):
        e = c
        te = toks[e][: CAP if ov else None]
        xs = xf[te] * p[te, None]  # fold gate prob into activations
        m = {
            "xq": _pack_xq(xs),
            "xt": _pack_tiles(xs, MTx - 2, H, t0_tok=2 * P),
            "w": W[e].astype(BF16),
        }
        if ov:
            g, h = c // 2, c % 2
            if g < len(pieces):
                e2, t2 = pieces[g]
                xs2 = (xf[t2] * p[t2, None])[:, h * KH : (h + 1) * KH]
                m["xt2"] = _pack_tiles(xs2, 1, KH)
                m["w2"] = W[e2][h * KH : (h + 1) * KH].astype(BF16)
            else:
                m["xt2"] = np.zeros((P, KH), dtype=BF16)
                m["w2"] = np.zeros((KH, H), dtype=BF16)
        in_maps.append(m)

    res = run_bass_kernel_spmd(
        nc,
        in_maps,
        core_ids=list(range(N_CORES)),
        trace=trace,
        trace_cores=list(range(N_CORES)) if trace else None,
    )

    out = np.empty((T, H), dtype=np.float32)
    for e in range(E):
        te = toks[e][: CAP if ov else None]
        ye = res.results[e]["y"][: len(te)].astype(np.float32)
        if np.any(b[e]):
            ye = ye + p[te, None] * b[e]
        out[te] = ye
    if ov:
        for g in range(len(pieces)):
            e2, t2 = pieces[g]
            ye = (
                res.results[2 * g]["y2"][: len(t2)].astype(np.float32)
                + res.results[2 * g + 1]["y2"][: len(t2)].astype(np.float32)
            )
            if np.any(b[e2]):
                ye = ye + p[t2, None] * b[e2]
            out[t2] = ye
    return out.reshape(B, S, H), res


def kernel(**inputs) -> np.ndarray:
    out, _ = _run(inputs, trace=os.environ.get("MOE_TRACE", "0") == "1")
    return out


def run_traced(inputs):
    """For test.py: returns (output, BassKernelResults with exec_time_ns)."""
    return _run(inputs, trace=True)
